# revision 25
# baseline (speedup 1.0000x reference)
"""BERT self-attention Bass kernel for 8 Trainium2 NeuronCores.

Problem: hidden_states [2, 2048, 768], 12 heads x 64 dim, fp32.

Sharding (zero-communication): core c in 0..7 handles batch b = c//4 and
head-group g = c%4 (3 heads). Host pre-lays inputs per core (fp16):
  - hsT   [768, 2048]  hidden[b].T
  - w     [768, 640]   QKV weight columns packed into 5 psum row-groups:
                       g0=[q0|q1] g1=[k0|k1] g2=[q2|v0] g3=[k2|v1] g4=[v2|0]
                       (pairing keeps each head's Q and K partition-aligned;
                       softmax 1/8 folded into Wq)
  - bias  [128, 5]     combined bias per row-group (fp32)
  - maskr [128, 17]    attention_mask[b] column-tiled (col i = keys
                       i*128..i*128+127); col 16 unused
  - ident [128, 128]   identity (PE transposes)

Device schedule (fp16 matmuls, fp32 psum accumulate), fully software-
pipelined so the first attention exp fires ~12us in and the ACT engine
(the 107us exp floor) is never starved afterwards:
  - hsT streams in per (kc-chunk, 512-col block) so the first s-block's
    QKV needs only ~1.7MB of DMA; DMA triggers are spread across the
    sync/gpsimd/scalar HWDGE rings with at most small transfers ahead of
    each ring's first compute (a ring's compute drains its DGE queue).
  - Warm-up matmuls on a memset tile keep the PE busy from engine-init
    so the HAM clock ramp (~2.4GHz after a few us continuous busy) is
    done before attention starts.
  - Prefix: g3-cb0 + g2-cb0 kc-major (k2/q2/v1 for queries+keys 0..511),
    then attention on head 2 begins immediately. All remaining QKV
    units (one unit = one (group, 512-col block), 6 accumulating
    matmuls), the V transposes (PE + DVE exp(mask)-fold into V_aug
    [t, 65] = exp(mask)*[V | 1]) are interleaved into attention-step
    slack with a deadline-derived static placement.
  - per (head, s-block of 512), heads 2,0,1: per t-pair step: 2 scores
    matmuls -> one exp over [128,1024] on ACT (unnormalized, no max-sub:
    scores are O(6) by construction) -> 2 PV matmuls accumulate
    ctxT[65, s] (denominator = ones column). PV emission is pipelined
    one step behind so the PE always has scores work while ACT runs exp.
Host: divide rows 0..63 by row 64, transpose to [s, d], interleave heads.
"""

import os

import numpy as np

import concourse.mybir as mybir
import concourse.tile as tile
from concourse import bacc
from concourse.bass_utils import run_bass_kernel_spmd

F32 = mybir.dt.float32
F16 = mybir.dt.float16

B = 2
S = 2048
HID = 768
NH = 12          # total heads
D = 64           # head dim
NHL = 3          # heads per core
DG = NHL * D     # 192 cols of each W per core
QKV = 640        # packed QKVT row space (5 groups of 128)
KC = HID // 128  # 6 contraction chunks
NG = 5           # psum row-groups of QKVT
GW = [128, 128, 128, 128, 64]     # real rows per group
NT = S // 128    # 16 key tiles
SBW = 512        # s-block width
NSB = S // SBW   # 4 s-blocks
NSTEP = NT // 2  # t-pair steps per s-block
NQ = NT // 4     # vaug quarters (4 t-tiles each)

# (group, offset) per quantity and head
QPOS = [(0, 0), (0, 64), (2, 0)]
KPOS = [(1, 0), (1, 64), (3, 0)]
VPOS = [(2, 64), (3, 64), (4, 0)]
HORDER = [2, 0, 1]  # head 2's tiles are ready first

LAST_EXEC_TIME_NS = None

_CACHED_NC = None


def _build_nc():
    nc = bacc.Bacc("TRN2", target_bir_lowering=False, debug=False, num_devices=8)

    hsT_d = nc.dram_tensor("hsT", [HID, S], F16, kind="ExternalInput")
    # w is host-relaid as [128, KC, 640] (partition-major) so the whole
    # tensor loads with one trigger of 128 contiguous 7.5KB descriptors —
    # DMA throughput is descriptor-bound below ~4KB/descriptor
    w_d = nc.dram_tensor("w", [128, KC * QKV], F16, kind="ExternalInput")
    bias_d = nc.dram_tensor("bias", [128, NG], F32, kind="ExternalInput")
    maskr_d = nc.dram_tensor("maskr", [128, NT + 1], F32, kind="ExternalInput")
    ident_d = nc.dram_tensor("ident", [128, 128], F16, kind="ExternalInput")
    out_d = nc.dram_tensor("ctxa", [NHL, D + 1, S], F32, kind="ExternalOutput")

    with tile.TileContext(nc) as tc:
        with (
            tc.tile_pool(name="const", bufs=1) as cp,
            tc.tile_pool(name="qkvt", bufs=1) as qp,
            tc.tile_pool(name="vaug", bufs=1) as vp,
            tc.tile_pool(name="probs", bufs=3) as pp,
            tc.tile_pool(name="oc", bufs=3) as op,
            tc.tile_pool(name="ps_sc", bufs=2, space="PSUM") as ps_sc,
            tc.tile_pool(name="ps_cx", bufs=2, space="PSUM") as ps_cx,
            tc.tile_pool(name="ps_ac", bufs=2, space="PSUM") as ps_ac,
        ):
            # --- SBUF tiles ---
            w_sb = cp.tile([128, KC, QKV], F16, tag="w")
            w_ap = w_d.ap().rearrange("p (kc n) -> p kc n", kc=KC)
            bias_sb = cp.tile([128, NG], F32, tag="bias")
            maskr_sb = cp.tile([128, NT + 1], F32, tag="maskr")
            ident = cp.tile([128, 128], F16, tag="ident")
            wfsrc = cp.tile([128, 16], F16, tag="wfsrc")
            hs = [
                cp.tile([128, S], F16, tag=f"hsT{kc}", name=f"hsT{kc}")
                for kc in range(KC)
            ]
            qkvt = [
                qp.tile([128, S], F16, tag=f"qkvt{g}", name=f"qkvt{g}")
                for g in range(NG)
            ]
            # K weights per head in [128, S] tiles with the other 64
            # partitions zeroed: scores matmuls then contract over K=128
            # (the zero rows multiply whatever sits in the rhs partitions
            # and contribute nothing)
            ktp = [
                qp.tile([128, S], F16, tag=f"ktp{h}", name=f"ktp{h}")
                for h in range(NHL)
            ]
            vaug = [
                vp.tile([128, NT, D + 1], F16, tag=f"vaug{h}", name=f"vaug{h}")
                for h in range(NHL)
            ]
            em = cp.tile([128, NT], F32, tag="em")

            # --- DMA triggers, spread across all three HWDGE rings ---
            # (each ring's queue sustains only ~110GB/s, so the 4.1MB of
            # input needs all three to land in ~13us). sync: hsT chunks
            # 0,2,4 (no compute on sync). gpsimd: wfsrc memset first (PE
            # warm-up must not wait), then the w thirds and chunk 5.
            # scalar: chunks 1,3 + the small tensors — they all land
            # before ACT's first exp needs its DGE drain.
            nc.gpsimd.memset(wfsrc[:], 1.0)
            for j in range(3):
                nc.gpsimd.dma_start(
                    w_sb[:, 2 * j : 2 * j + 2, :], w_ap[:, 2 * j : 2 * j + 2, :]
                )
            nc.scalar.dma_start(bias_sb[:], bias_d.ap())
            nc.scalar.dma_start(maskr_sb[:], maskr_d.ap())
            nc.scalar.dma_start(ident[:], ident_d.ap())
            RING = {0: nc.sync, 1: nc.scalar, 2: nc.sync, 3: nc.scalar,
                    4: nc.gpsimd, 5: nc.sync}
            for kc in range(KC):
                RING[kc].dma_start(
                    hs[kc][:], hsT_d.ap()[kc * 128 : (kc + 1) * 128, :]
                )

            # zero fill for ktp[2]: DVE queue head, done long before the
            # h2 scores need it; ktp[0]/ktp[1] zeroes are deferred into
            # interleave closures so they don't delay the prefix copies
            nc.vector.memset(ktp[2][64:128, :], 0.0)

            # --- PE warm-up: keep the PE busy from engine-init onward so
            # the HAM clock ramp completes before real work piles up ---
            def warm(n):
                for _ in range(n):
                    wf = ps_ac.tile([128, SBW], F32, tag="acc", name="wf")
                    nc.tensor.matmul(
                        wf[:16, :16],
                        wfsrc[:],
                        wfsrc[:],
                        start=True,
                        stop=True,
                        skip_group_check=True,
                    )

            # --- QKV units: one unit = (group gi, column block cb), six
            # accumulating matmuls over the kc chunks + psum->sbuf copy
            # (DVE, bias fused) on the last. acc_of overrides the psum
            # accumulator (the prefix borrows the idle sc-pool banks). ---
            KORD = [0, 1, 2, 3, 4, 5]  # expected chunk-arrival order

            def qkv_unit(gi, cb, acc_of=None, act_copy=False):
                gw = GW[gi]
                st8 = {}

                for i in range(KC):

                    def mm(i=i, kc=KORD[i], gi=gi, cb=cb, gw=gw):
                        if i == 0:
                            st8["acc"] = (
                                acc_of()
                                if acc_of is not None
                                else ps_ac.tile(
                                    [128, SBW], F32, tag="acc", name="acc"
                                )
                            )
                        acc = st8["acc"]
                        nc.tensor.matmul(
                            acc[:gw, :],
                            w_sb[:, kc, gi * 128 : gi * 128 + gw],
                            hs[kc][:, cb * SBW : (cb + 1) * SBW],
                            start=(i == 0),
                            stop=(i == KC - 1),
                        )
                        if i == KC - 1:
                            sl = slice(cb * SBW, (cb + 1) * SBW)

                            def cp_(dst, rows, b0):
                                bias_ap = bias_sb[
                                    b0 : b0 + (rows.stop - rows.start),
                                    gi : gi + 1,
                                ]
                                if act_copy:
                                    # ACT is idle before the first exp —
                                    # run this copy there, in parallel
                                    # with the DVE copies
                                    nc.scalar.activation(
                                        dst[rows, sl],
                                        acc[rows, :],
                                        mybir.ActivationFunctionType.Identity,
                                        bias=bias_ap,
                                    )
                                else:
                                    nc.vector.tensor_scalar_add(
                                        dst[rows, sl],
                                        acc[rows, :],
                                        bias_ap,
                                    )

                            lo, hi = slice(0, 64), slice(64, 128)
                            if gi == 0:  # q0|q1
                                cp_(qkvt[0], slice(0, 128), 0)
                            elif gi == 1:  # k0|k1 -> ktp
                                cp_(ktp[0], lo, 0)
                                cp_(ktp[1], hi, 64)
                            elif gi == 2:  # q2|v0
                                cp_(qkvt[2], slice(0, 128), 0)
                            elif gi == 3:  # k2|v1
                                cp_(ktp[2], lo, 0)
                                cp_(qkvt[3], hi, 64)
                            else:  # v2
                                cp_(qkvt[4], lo, 0)

                    yield mm

            # --- V_aug quarter: 4 PE transposes + DVE exp(mask) fold ---
            def vaug_quarter(h, q):
                def unit(h=h, q=q):
                    ti, off = VPOS[h]
                    vt = qkvt[ti]
                    tp = ps_ac.tile([128, 4 * D], F16, tag="acc", name="tp")
                    for j in range(4):
                        tt = 4 * q + j
                        nc.tensor.transpose(
                            tp[:, j * D : (j + 1) * D],
                            vt[off : off + D, tt * 128 : (tt + 1) * 128],
                            ident[off : off + D, off : off + D],
                        )
                    emq = em[:, 4 * q : 4 * (q + 1)]
                    nc.vector.tensor_tensor(
                        vaug[h][:, 4 * q : 4 * (q + 1), :D],
                        tp[:].rearrange("p (j d) -> p j d", d=D),
                        emq.rearrange("p (j o) -> p j o", o=1)
                        .broadcast_to([128, 4, D]),
                        mybir.AluOpType.mult,
                    )
                    nc.vector.tensor_copy(
                        vaug[h][:, 4 * q : 4 * (q + 1), D : D + 1],
                        emq.rearrange("p (j o) -> p j o", o=1),
                    )

                return unit

            # em[t] = exp(mask_t), folded into V_aug (ACT; before the
            # attention exps in the ACT queue)
            nc.scalar.activation(
                em[:], maskr_sb[:, :NT], mybir.ActivationFunctionType.Exp
            )

            warm(30)

            # --- prefix: ALL FIVE groups of column-block 0 plus three
            # cb1 "floaters", kc-major (the PE chews each chunk faster
            # than the two DMA queues stream them in, so this whole phase
            # is DMA-paced and the HAM clock ramp completes before
            # attention; the floaters soak up the leftover PE idle and
            # empty h2's sb1 production out of the attention steps).
            # Accumulators borrow every idle psum bank: scA = g3|g2,
            # scB = g4|g0, acc-tag = g1 + g2cb1, ctx-tag = g3cb1 + g4cb1.
            scA = ps_sc.tile([128, 2 * SBW], F32, tag="sc", name="scA")
            scB = ps_sc.tile([128, 2 * SBW], F32, tag="sc", name="scB")

            def ctx_acc():
                return ps_cx.tile([128, SBW], F32, tag="ctx", name="ctx")

            pref = [
                list(qkv_unit(3, 0, lambda: scA[:, :SBW])),
                list(qkv_unit(2, 0, lambda: scA[:, SBW:], act_copy=True)),
                list(qkv_unit(4, 0, lambda: scB[:, :SBW])),
                list(qkv_unit(0, 0, lambda: scB[:, SBW:])),
                list(qkv_unit(1, 0)),
                list(qkv_unit(3, 1, ctx_acc)),
                list(qkv_unit(4, 1, ctx_acc)),
                list(qkv_unit(2, 1)),
            ]
            NWARM = [5, 3, 5, 0, 0, 0]  # ramp-hold fillers per chunk stall
            for i in range(KC):
                for u in pref:
                    u[i]()
                warm(NWARM[i])

            # --- interleave plan: every remaining unit gets an emission
            # window [d0, d1] in global attention steps (96 = 3 heads x 4
            # s-blocks x 8 t-pair steps); closures are spread over the
            # window. Windows front-load just enough to meet each
            # consumer's first-use step (h2 needs all K2/V2 by step 7,
            # q2-sb(k) by step 8k; h0 at steps 32.. needs g0/g1/T0;
            # h1 at 64.. needs only q1 (g0) and T1 which are relaxed). ---
            plan = []

            def add(closures, d0, d1):
                plan.append((list(closures), d0, d1))

            # NOTE: a unit whose output feeds the scores matmul of step s
            # must have d1 <= s-1 (scores of step s are EMITTED before
            # sched[s] runs; a later write would create no dep edge and
            # the scores would read garbage) — and d1 <= s-2 where slack
            # allows, to hide the DVE psum->sbuf copy latency. PV
            # consumers of step st are emitted at step st+1's flush,
            # after sched[st+1].
            def ktp_zero(h, rows, c0):
                def unit(h=h, rows=rows, c0=c0):
                    nc.vector.memset(ktp[h][rows, c0 : c0 + S // 2], 0.0)

                return unit

            add([vaug_quarter(2, 0)], 0, 0)
            add([vaug_quarter(2, 1)], 1, 1)
            add(qkv_unit(3, 2), 0, 2)
            add(qkv_unit(4, 2), 2, 3)
            add([vaug_quarter(2, 2)], 4, 4)
            add(qkv_unit(3, 3), 3, 4)
            add(qkv_unit(4, 3), 4, 5)
            add([vaug_quarter(2, 3)], 6, 6)
            add(qkv_unit(2, 2), 7, 13)
            add(qkv_unit(2, 3), 14, 20)
            add([ktp_zero(0, slice(64, 128), 0),
                 ktp_zero(0, slice(64, 128), S // 2),
                 ktp_zero(1, slice(0, 64), 0),
                 ktp_zero(1, slice(0, 64), S // 2)], 21, 27)
            add([vaug_quarter(0, 0)], 30, 31)
            add(qkv_unit(1, 1), 31, 32)
            add([vaug_quarter(0, 1)], 33, 33)
            add(qkv_unit(1, 2), 33, 34)
            add([vaug_quarter(0, 2)], 35, 35)
            add(qkv_unit(1, 3), 35, 36)
            add([vaug_quarter(0, 3)], 37, 37)
            add(qkv_unit(0, 1), 37, 38)
            add(qkv_unit(0, 2), 42, 46)
            add(qkv_unit(0, 3), 50, 54)
            add([vaug_quarter(1, 0)], 58, 58)
            add([vaug_quarter(1, 1)], 60, 60)
            add([vaug_quarter(1, 2)], 62, 62)
            add([vaug_quarter(1, 3)], 64, 64)

            sched = [[] for _ in range(96)]
            for closures, d0, d1 in plan:
                span = d1 - d0 + 1
                for i, c in enumerate(closures):
                    g = d0 + (i * span) // len(closures)
                    sched[g].append(c)

            # --- attention ---
            # software-pipelined emission: the PV pair of step k is emitted
            # AFTER exp(k+1), so in the PE's static order the next scores
            # pair runs while ACT is busy and ACT never waits on the PE.
            pending = [None]  # (h, ctx, st, pr, oc_args) awaiting PV

            def flush_pending():
                if pending[0] is None:
                    return
                (ph, pctx, pst, ppr, poc) = pending[0]
                for half in range(2):
                    tt = pst * 2 + half
                    nc.tensor.matmul(
                        pctx[: D + 1, :],
                        vaug[ph][:, tt, :],
                        ppr[:, half * SBW : (half + 1) * SBW],
                        start=(tt == 0),
                        stop=(tt == NT - 1),
                    )
                if poc is not None:
                    h_, s0_ = poc
                    oc = op.tile([128, SBW], F32, tag="oc", name="oc")
                    if h_ == HORDER[-1] and s0_ == S - SBW:
                        # final s-block: copy + DMA in halves so the DMA
                        # descriptor-gen overlaps the second copy half
                        for c0 in (0, SBW // 2):
                            hsl = slice(c0, c0 + SBW // 2)
                            nc.vector.tensor_copy(
                                oc[: D + 1, hsl], pctx[: D + 1, hsl]
                            )
                            nc.sync.dma_start(
                                out_d.ap()[h_, :, s0_ + c0 : s0_ + c0 + SBW // 2],
                                oc[: D + 1, hsl],
                            )
                    else:
                        nc.vector.tensor_copy(oc[: D + 1, :], pctx[: D + 1, :])
                        nc.sync.dma_start(
                            out_d.ap()[h_, :, s0_ : s0_ + SBW],
                            oc[: D + 1, :],
                        )

            QTILE = [0, 0, 2]  # rhs tile per head (full 128 partitions)
            gstep = 0
            for h in HORDER:
                qt, kt = qkvt[QTILE[h]], ktp[h]
                for sbk in range(NSB):
                    s0 = sbk * SBW
                    ctx = ps_cx.tile([128, SBW], F32, tag="ctx", name="ctx")
                    for st in range(NSTEP):  # t-pair steps
                        sc = ps_sc.tile(
                            [128, 2 * SBW], F32, tag="sc", name="sc"
                        )
                        for half in range(2):
                            tt = st * 2 + half
                            nc.tensor.matmul(
                                sc[:, half * SBW : (half + 1) * SBW],
                                kt[:, tt * 128 : (tt + 1) * 128],
                                qt[:, s0 : s0 + SBW],
                                start=True,
                                stop=True,
                            )
                        for c in sched[gstep]:
                            c()
                        pr = pp.tile([128, 2 * SBW], F16, tag="pr", name="pr")
                        nc.scalar.activation(
                            pr[:], sc[:], mybir.ActivationFunctionType.Exp
                        )
                        flush_pending()
                        pending[0] = (
                            h,
                            ctx,
                            st,
                            pr,
                            (h, s0) if st == NSTEP - 1 else None,
                        )
                        gstep += 1
            flush_pending()

    nc.compile()
    return nc


def _get_nc():
    global _CACHED_NC
    if _CACHED_NC is None:
        _CACHED_NC = _build_nc()
    return _CACHED_NC


def kernel(
    hidden_states, attention_mask, Wq, bq, Wk, bk, Wv, bv
) -> np.ndarray:
    global LAST_EXEC_TIME_NS
    hidden_states = np.asarray(hidden_states, dtype=np.float32)
    attention_mask = np.asarray(attention_mask, dtype=np.float32)
    Wq = np.asarray(Wq, dtype=np.float32)
    Wk = np.asarray(Wk, dtype=np.float32)
    Wv = np.asarray(Wv, dtype=np.float32)
    bq = np.asarray(bq, dtype=np.float32)
    bk = np.asarray(bk, dtype=np.float32)
    bv = np.asarray(bv, dtype=np.float32)

    scale = 1.0 / np.sqrt(np.float32(D))

    in_maps = []
    for c in range(8):
        b, g = divmod(c, 4)
        cols = slice(g * DG, (g + 1) * DG)
        wq = Wq[:, cols] * scale
        wk = Wk[:, cols]
        wv = Wv[:, cols]
        w = np.zeros((HID, QKV), dtype=np.float32)
        bcat = np.zeros(QKV, dtype=np.float32)
        bq_, bk_, bv_ = bq[cols] * scale, bk[cols], bv[cols]
        for h in range(NHL):
            for (pos, mat, bb) in (
                (QPOS[h], wq, bq_),
                (KPOS[h], wk, bk_),
                (VPOS[h], wv, bv_),
            ):
                gi, off = pos
                r0 = gi * 128 + off
                w[:, r0 : r0 + D] = mat[:, h * D : (h + 1) * D]
                bcat[r0 : r0 + D] = bb[h * D : (h + 1) * D]
        bias = np.ascontiguousarray(bcat.reshape(NG, 128).T)
        maskr = np.zeros((128, NT + 1), dtype=np.float32)
        maskr[:, :NT] = attention_mask[b, 0, 0, :].reshape(NT, 128).T
        # partition-major relayout: w_host[p, kc*QKV + n] = w[kc*128+p, n]
        # so the device loads w with 128 contiguous 7.5KB descriptors
        w_host = np.ascontiguousarray(
            w.reshape(KC, 128, QKV).transpose(1, 0, 2).reshape(128, KC * QKV)
        )
        in_maps.append(
            {
                "hsT": np.ascontiguousarray(hidden_states[b].T).astype(np.float16),
                "w": w_host.astype(np.float16),
                "bias": bias,
                "maskr": maskr,
                "ident": np.eye(128, dtype=np.float16),
            }
        )

    nc = _get_nc()
    trace = bool(os.environ.get("BASS_KERNEL_TRACE"))
    res = run_bass_kernel_spmd(nc, in_maps, list(range(8)), trace=trace)
    LAST_EXEC_TIME_NS = res.exec_time_ns

    out = np.empty((B, S, HID), dtype=np.float32)
    for c in range(8):
        b, g = divmod(c, 4)
        ctxa = res.results[c]["ctxa"]  # [3, 65, 2048]
        for hl in range(NHL):
            ctx = ctxa[hl, :D, :] / ctxa[hl, D : D + 1, :]  # [64, 2048]
            out[b, :, g * DG + hl * D : g * DG + (hl + 1) * D] = ctx.T
    return out


# revision 26
# speedup vs baseline: 1.0573x; 1.0573x over previous
"""BERT self-attention Bass kernel for 8 Trainium2 NeuronCores.

Problem: hidden_states [2, 2048, 768], 12 heads x 64 dim, fp32.

Sharding (zero-communication): core c in 0..7 handles batch b = c//4 and
head-group g = c%4 (3 heads). Host pre-lays inputs per core (fp16):
  - hsT   [768, 2048]  hidden[b].T
  - w     [768, 640]   QKV weight columns packed into 5 psum row-groups:
                       g0=[q0|q1] g1=[k0|k1] g2=[q2|v0] g3=[k2|v1] g4=[v2|0]
                       (pairing keeps each head's Q and K partition-aligned;
                       softmax 1/8 folded into Wq)
  - bias  [128, 5]     combined bias per row-group (fp32)
  - maskr [128, 17]    attention_mask[b] column-tiled (col i = keys
                       i*128..i*128+127); col 16 unused
  - ident [128, 128]   identity (PE transposes)

Device schedule (fp16 matmuls, fp32 psum accumulate), fully software-
pipelined so the first attention exp fires ~12us in and the ACT engine
(the 107us exp floor) is never starved afterwards:
  - hsT streams in per (kc-chunk, 512-col block) so the first s-block's
    QKV needs only ~1.7MB of DMA; DMA triggers are spread across the
    sync/gpsimd/scalar HWDGE rings with at most small transfers ahead of
    each ring's first compute (a ring's compute drains its DGE queue).
  - Warm-up matmuls on a memset tile keep the PE busy from engine-init
    so the HAM clock ramp (~2.4GHz after a few us continuous busy) is
    done before attention starts.
  - Prefix: g3-cb0 + g2-cb0 kc-major (k2/q2/v1 for queries+keys 0..511),
    then attention on head 2 begins immediately. All remaining QKV
    units (one unit = one (group, 512-col block), 6 accumulating
    matmuls), the V transposes (PE + DVE exp(mask)-fold into V_aug
    [t, 65] = exp(mask)*[V | 1]) are interleaved into attention-step
    slack with a deadline-derived static placement.
  - per (head, s-block of 512), heads 2,0,1: per t-pair step: 2 scores
    matmuls -> one exp over [128,1024] on ACT (unnormalized, no max-sub:
    scores are O(6) by construction) -> 2 PV matmuls accumulate
    ctxT[65, s] (denominator = ones column). PV emission is pipelined
    one step behind so the PE always has scores work while ACT runs exp.
Host: divide rows 0..63 by row 64, transpose to [s, d], interleave heads.
"""

import os

import numpy as np

import concourse.mybir as mybir
import concourse.tile as tile
from concourse import bacc
from concourse.bass_utils import run_bass_kernel_spmd

F32 = mybir.dt.float32
F16 = mybir.dt.float16

B = 2
S = 2048
HID = 768
NH = 12          # total heads
D = 64           # head dim
NHL = 3          # heads per core
DG = NHL * D     # 192 cols of each W per core
QKV = 640        # packed QKVT row space (5 groups of 128)
KC = HID // 128  # 6 contraction chunks
NG = 5           # psum row-groups of QKVT
GW = [128, 128, 128, 128, 64]     # real rows per group
NT = S // 128    # 16 key tiles
SBW = 512        # s-block width
NSB = S // SBW   # 4 s-blocks
NSTEP = NT // 2  # t-pair steps per s-block
NQ = NT // 4     # vaug quarters (4 t-tiles each)

# (group, offset) per quantity and head
QPOS = [(0, 0), (0, 64), (2, 0)]
KPOS = [(1, 0), (1, 64), (3, 0)]
VPOS = [(2, 64), (3, 64), (4, 0)]
HORDER = [2, 0, 1]  # head 2's tiles are ready first

LAST_EXEC_TIME_NS = None

_CACHED_NC = None


def _build_nc():
    nc = bacc.Bacc("TRN2", target_bir_lowering=False, debug=False, num_devices=8)

    hsT_d = nc.dram_tensor("hsT", [HID, S], F16, kind="ExternalInput")
    # w is host-relaid as [128, KC, 640] (partition-major) so the whole
    # tensor loads with one trigger of 128 contiguous 7.5KB descriptors —
    # DMA throughput is descriptor-bound below ~4KB/descriptor
    w_d = nc.dram_tensor("w", [128, KC * QKV], F16, kind="ExternalInput")
    bias_d = nc.dram_tensor("bias", [128, NG], F32, kind="ExternalInput")
    maskr_d = nc.dram_tensor("maskr", [128, NT + 1], F32, kind="ExternalInput")
    ident_d = nc.dram_tensor("ident", [128, 128], F16, kind="ExternalInput")
    out_d = nc.dram_tensor("ctxa", [NHL, D + 1, S], F32, kind="ExternalOutput")

    with tile.TileContext(nc) as tc:
        with (
            tc.tile_pool(name="const", bufs=1) as cp,
            tc.tile_pool(name="qkvt", bufs=1) as qp,
            tc.tile_pool(name="vaug", bufs=1) as vp,
            tc.tile_pool(name="probs", bufs=3) as pp,
            tc.tile_pool(name="oc", bufs=3) as op,
            tc.tile_pool(name="ps_sc", bufs=2, space="PSUM") as ps_sc,
            tc.tile_pool(name="ps_cx", bufs=2, space="PSUM") as ps_cx,
            tc.tile_pool(name="ps_ac", bufs=2, space="PSUM") as ps_ac,
        ):
            # --- SBUF tiles ---
            w_sb = cp.tile([128, KC, QKV], F16, tag="w")
            w_ap = w_d.ap().rearrange("p (kc n) -> p kc n", kc=KC)
            bias_sb = cp.tile([128, NG], F32, tag="bias")
            maskr_sb = cp.tile([128, NT + 1], F32, tag="maskr")
            ident = cp.tile([128, 128], F16, tag="ident")
            wfsrc = cp.tile([128, 16], F16, tag="wfsrc")
            hs = [
                cp.tile([128, S], F16, tag=f"hsT{kc}", name=f"hsT{kc}")
                for kc in range(KC)
            ]
            qkvt = [
                qp.tile([128, S], F16, tag=f"qkvt{g}", name=f"qkvt{g}")
                for g in range(NG)
            ]
            # K weights per head in [128, S] tiles with the other 64
            # partitions zeroed: scores matmuls then contract over K=128
            # (the zero rows multiply whatever sits in the rhs partitions
            # and contribute nothing)
            ktp = [
                qp.tile([128, S], F16, tag=f"ktp{h}", name=f"ktp{h}")
                for h in range(NHL)
            ]
            vaug = [
                vp.tile([128, NT, D + 1], F16, tag=f"vaug{h}", name=f"vaug{h}")
                for h in range(NHL)
            ]
            em = cp.tile([128, NT], F32, tag="em")

            # --- DMA triggers, spread across all three HWDGE rings ---
            # (each ring's queue sustains only ~110GB/s, so the 4.1MB of
            # input needs all three to land in ~13us). sync: hsT chunks
            # 0,2,4 (no compute on sync). gpsimd: wfsrc memset first (PE
            # warm-up must not wait), then the w thirds and chunk 5.
            # scalar: chunks 1,3 + the small tensors — they all land
            # before ACT's first exp needs its DGE drain.
            nc.gpsimd.memset(wfsrc[:], 1.0)
            for j in range(3):
                nc.gpsimd.dma_start(
                    w_sb[:, 2 * j : 2 * j + 2, :], w_ap[:, 2 * j : 2 * j + 2, :]
                )
            RING = {0: nc.sync, 1: nc.sync, 2: nc.sync, 3: nc.gpsimd,
                    4: nc.sync, 5: nc.gpsimd}
            for kc in range(KC):
                RING[kc].dma_start(
                    hs[kc][:], hsT_d.ap()[kc * 128 : (kc + 1) * 128, :]
                )
            nc.scalar.dma_start(bias_sb[:], bias_d.ap())
            nc.scalar.dma_start(maskr_sb[:], maskr_d.ap())
            nc.scalar.dma_start(ident[:], ident_d.ap())

            # zero fill for ktp[2]: DVE queue head, done long before the
            # h2 scores need it; ktp[0]/ktp[1] zeroes are deferred into
            # interleave closures so they don't delay the prefix copies
            nc.vector.memset(ktp[2][64:128, :], 0.0)

            # --- PE warm-up: keep the PE busy from engine-init onward so
            # the HAM clock ramp completes before real work piles up ---
            def warm(n):
                for _ in range(n):
                    wf = ps_ac.tile([128, SBW], F32, tag="acc", name="wf")
                    nc.tensor.matmul(
                        wf[:16, :16],
                        wfsrc[:],
                        wfsrc[:],
                        start=True,
                        stop=True,
                        skip_group_check=True,
                    )

            # --- QKV units: one unit = (group gi, column block cb), six
            # accumulating matmuls over the kc chunks + psum->sbuf copy
            # (DVE, bias fused) on the last. acc_of overrides the psum
            # accumulator (the prefix borrows the idle sc-pool banks). ---
            KORD = [0, 1, 2, 3, 4, 5]  # expected chunk-arrival order

            def qkv_unit(gi, cb, acc_of=None, act_copy=""):
                gw = GW[gi]
                st8 = {}

                for i in range(KC):

                    def mm(i=i, kc=KORD[i], gi=gi, cb=cb, gw=gw):
                        if i == 0:
                            st8["acc"] = (
                                acc_of()
                                if acc_of is not None
                                else ps_ac.tile(
                                    [128, SBW], F32, tag="acc", name="acc"
                                )
                            )
                        acc = st8["acc"]
                        nc.tensor.matmul(
                            acc[:gw, :],
                            w_sb[:, kc, gi * 128 : gi * 128 + gw],
                            hs[kc][:, cb * SBW : (cb + 1) * SBW],
                            start=(i == 0),
                            stop=(i == KC - 1),
                        )
                        if i == KC - 1:
                            sl = slice(cb * SBW, (cb + 1) * SBW)

                            def cp_(dst, rows, b0, which=""):
                                bias_ap = bias_sb[
                                    b0 : b0 + (rows.stop - rows.start),
                                    gi : gi + 1,
                                ]
                                if which in act_copy and which:
                                    # ACT is idle before the first exp —
                                    # run this copy there, in parallel
                                    # with the DVE copies
                                    nc.scalar.activation(
                                        dst[rows, sl],
                                        acc[rows, :],
                                        mybir.ActivationFunctionType.Identity,
                                        bias=bias_ap,
                                    )
                                else:
                                    nc.vector.tensor_scalar_add(
                                        dst[rows, sl],
                                        acc[rows, :],
                                        bias_ap,
                                    )

                            lo, hi = slice(0, 64), slice(64, 128)
                            if gi == 0:  # q0|q1
                                cp_(qkvt[0], slice(0, 128), 0, "q")
                            elif gi == 1:  # k0|k1 -> ktp
                                cp_(ktp[0], lo, 0, "k")
                                cp_(ktp[1], hi, 64, "k")
                            elif gi == 2:  # q2|v0
                                cp_(qkvt[2], slice(0, 128), 0, "q")
                            elif gi == 3:  # k2|v1
                                cp_(ktp[2], lo, 0, "k")
                                cp_(qkvt[3], hi, 64, "v")
                            else:  # v2
                                cp_(qkvt[4], lo, 0, "v")

                    yield mm

            # --- V_aug quarter: 4 PE transposes + DVE exp(mask) fold ---
            def vaug_quarter(h, q):
                def unit(h=h, q=q):
                    ti, off = VPOS[h]
                    vt = qkvt[ti]
                    tp = ps_ac.tile([128, 4 * D], F16, tag="acc", name="tp")
                    for j in range(4):
                        tt = 4 * q + j
                        nc.tensor.transpose(
                            tp[:, j * D : (j + 1) * D],
                            vt[off : off + D, tt * 128 : (tt + 1) * 128],
                            ident[off : off + D, off : off + D],
                        )
                    emq = em[:, 4 * q : 4 * (q + 1)]
                    nc.vector.tensor_tensor(
                        vaug[h][:, 4 * q : 4 * (q + 1), :D],
                        tp[:].rearrange("p (j d) -> p j d", d=D),
                        emq.rearrange("p (j o) -> p j o", o=1)
                        .broadcast_to([128, 4, D]),
                        mybir.AluOpType.mult,
                    )
                    nc.vector.tensor_copy(
                        vaug[h][:, 4 * q : 4 * (q + 1), D : D + 1],
                        emq.rearrange("p (j o) -> p j o", o=1),
                    )

                return unit

            # em[t] = exp(mask_t), folded into V_aug (ACT; before the
            # attention exps in the ACT queue)
            nc.scalar.activation(
                em[:], maskr_sb[:, :NT], mybir.ActivationFunctionType.Exp
            )

            warm(30)

            # --- prefix: ALL FIVE groups of column-block 0 plus three
            # cb1 "floaters", kc-major (the PE chews each chunk faster
            # than the two DMA queues stream them in, so this whole phase
            # is DMA-paced and the HAM clock ramp completes before
            # attention; the floaters soak up the leftover PE idle and
            # empty h2's sb1 production out of the attention steps).
            # Accumulators borrow every idle psum bank: scA = g3|g2,
            # scB = g4|g0, acc-tag = g1 + g2cb1, ctx-tag = g3cb1 + g4cb1.
            scA = ps_sc.tile([128, 2 * SBW], F32, tag="sc", name="scA")
            scB = ps_sc.tile([128, 2 * SBW], F32, tag="sc", name="scB")

            def ctx_acc():
                return ps_cx.tile([128, SBW], F32, tag="ctx", name="ctx")

            pref = [
                list(qkv_unit(3, 0, lambda: scA[:, :SBW], act_copy="v")),
                list(qkv_unit(2, 0, lambda: scA[:, SBW:])),
                list(qkv_unit(4, 0, lambda: scB[:, :SBW])),
                list(qkv_unit(0, 0, lambda: scB[:, SBW:])),
                list(qkv_unit(1, 0)),
                list(qkv_unit(3, 1, ctx_acc)),
                list(qkv_unit(4, 1, ctx_acc)),
                list(qkv_unit(2, 1)),
            ]
            for i in range(KC):
                for u in pref:
                    u[i]()

            # --- interleave plan: every remaining unit gets an emission
            # window [d0, d1] in global attention steps (96 = 3 heads x 4
            # s-blocks x 8 t-pair steps); closures are spread over the
            # window. Windows front-load just enough to meet each
            # consumer's first-use step (h2 needs all K2/V2 by step 7,
            # q2-sb(k) by step 8k; h0 at steps 32.. needs g0/g1/T0;
            # h1 at 64.. needs only q1 (g0) and T1 which are relaxed). ---
            plan = []

            def add(closures, d0, d1):
                plan.append((list(closures), d0, d1))

            # NOTE: a unit whose output feeds the scores matmul of step s
            # must have d1 <= s-1 (scores of step s are EMITTED before
            # sched[s] runs; a later write would create no dep edge and
            # the scores would read garbage) — and d1 <= s-2 where slack
            # allows, to hide the DVE psum->sbuf copy latency. PV
            # consumers of step st are emitted at step st+1's flush,
            # after sched[st+1].
            def ktp_zero(h, rows, c0):
                def unit(h=h, rows=rows, c0=c0):
                    nc.vector.memset(ktp[h][rows, c0 : c0 + S // 2], 0.0)

                return unit

            add([vaug_quarter(2, 0)], 0, 0)
            add([vaug_quarter(2, 1)], 1, 1)
            add(qkv_unit(3, 2), 0, 2)
            add(qkv_unit(4, 2), 2, 3)
            add([vaug_quarter(2, 2)], 4, 4)
            add(qkv_unit(3, 3), 3, 4)
            add(qkv_unit(4, 3), 4, 5)
            add([vaug_quarter(2, 3)], 6, 6)
            add(qkv_unit(2, 2), 7, 13)
            add(qkv_unit(2, 3), 14, 20)
            add([ktp_zero(0, slice(64, 128), 0),
                 ktp_zero(0, slice(64, 128), S // 2),
                 ktp_zero(1, slice(0, 64), 0),
                 ktp_zero(1, slice(0, 64), S // 2)], 21, 27)
            add([vaug_quarter(0, 0)], 30, 31)
            add(qkv_unit(1, 1), 31, 32)
            add([vaug_quarter(0, 1)], 33, 33)
            add(qkv_unit(1, 2), 33, 34)
            add([vaug_quarter(0, 2)], 35, 35)
            add(qkv_unit(1, 3), 35, 36)
            add([vaug_quarter(0, 3)], 37, 37)
            add(qkv_unit(0, 1), 37, 38)
            add(qkv_unit(0, 2), 42, 46)
            add(qkv_unit(0, 3), 50, 54)
            add([vaug_quarter(1, 0)], 58, 58)
            add([vaug_quarter(1, 1)], 60, 60)
            add([vaug_quarter(1, 2)], 62, 62)
            add([vaug_quarter(1, 3)], 64, 64)

            sched = [[] for _ in range(96)]
            for closures, d0, d1 in plan:
                span = d1 - d0 + 1
                for i, c in enumerate(closures):
                    g = d0 + (i * span) // len(closures)
                    sched[g].append(c)

            # --- attention ---
            # software-pipelined emission: the PV pair of step k is emitted
            # AFTER exp(k+1), so in the PE's static order the next scores
            # pair runs while ACT is busy and ACT never waits on the PE.
            pending = [None]  # (h, ctx, st, pr, oc_args) awaiting PV

            def flush_pending():
                if pending[0] is None:
                    return
                (ph, pctx, pst, ppr, poc) = pending[0]
                for half in range(2):
                    tt = pst * 2 + half
                    nc.tensor.matmul(
                        pctx[: D + 1, :],
                        vaug[ph][:, tt, :],
                        ppr[:, half * SBW : (half + 1) * SBW],
                        start=(tt == 0),
                        stop=(tt == NT - 1),
                    )
                if poc is not None:
                    h_, s0_ = poc
                    oc = op.tile([128, SBW], F32, tag="oc", name="oc")
                    if h_ == HORDER[-1] and s0_ == S - SBW:
                        # final s-block: copy + DMA in halves so the DMA
                        # descriptor-gen overlaps the second copy half
                        for c0 in (0, SBW // 2):
                            hsl = slice(c0, c0 + SBW // 2)
                            nc.vector.tensor_copy(
                                oc[: D + 1, hsl], pctx[: D + 1, hsl]
                            )
                            nc.sync.dma_start(
                                out_d.ap()[h_, :, s0_ + c0 : s0_ + c0 + SBW // 2],
                                oc[: D + 1, hsl],
                            )
                    else:
                        nc.vector.tensor_copy(oc[: D + 1, :], pctx[: D + 1, :])
                        nc.sync.dma_start(
                            out_d.ap()[h_, :, s0_ : s0_ + SBW],
                            oc[: D + 1, :],
                        )

            QTILE = [0, 0, 2]  # rhs tile per head (full 128 partitions)
            gstep = 0
            for h in HORDER:
                qt, kt = qkvt[QTILE[h]], ktp[h]
                for sbk in range(NSB):
                    s0 = sbk * SBW
                    ctx = ps_cx.tile([128, SBW], F32, tag="ctx", name="ctx")
                    for st in range(NSTEP):  # t-pair steps
                        sc = ps_sc.tile(
                            [128, 2 * SBW], F32, tag="sc", name="sc"
                        )
                        for half in range(2):
                            tt = st * 2 + half
                            nc.tensor.matmul(
                                sc[:, half * SBW : (half + 1) * SBW],
                                kt[:, tt * 128 : (tt + 1) * 128],
                                qt[:, s0 : s0 + SBW],
                                start=True,
                                stop=True,
                            )
                        for c in sched[gstep]:
                            c()
                        pr = pp.tile([128, 2 * SBW], F16, tag="pr", name="pr")
                        nc.scalar.activation(
                            pr[:], sc[:], mybir.ActivationFunctionType.Exp
                        )
                        flush_pending()
                        pending[0] = (
                            h,
                            ctx,
                            st,
                            pr,
                            (h, s0) if st == NSTEP - 1 else None,
                        )
                        gstep += 1
            flush_pending()

    nc.compile()
    return nc


def _get_nc():
    global _CACHED_NC
    if _CACHED_NC is None:
        _CACHED_NC = _build_nc()
    return _CACHED_NC


def kernel(
    hidden_states, attention_mask, Wq, bq, Wk, bk, Wv, bv
) -> np.ndarray:
    global LAST_EXEC_TIME_NS
    hidden_states = np.asarray(hidden_states, dtype=np.float32)
    attention_mask = np.asarray(attention_mask, dtype=np.float32)
    Wq = np.asarray(Wq, dtype=np.float32)
    Wk = np.asarray(Wk, dtype=np.float32)
    Wv = np.asarray(Wv, dtype=np.float32)
    bq = np.asarray(bq, dtype=np.float32)
    bk = np.asarray(bk, dtype=np.float32)
    bv = np.asarray(bv, dtype=np.float32)

    scale = 1.0 / np.sqrt(np.float32(D))

    in_maps = []
    for c in range(8):
        b, g = divmod(c, 4)
        cols = slice(g * DG, (g + 1) * DG)
        wq = Wq[:, cols] * scale
        wk = Wk[:, cols]
        wv = Wv[:, cols]
        w = np.zeros((HID, QKV), dtype=np.float32)
        bcat = np.zeros(QKV, dtype=np.float32)
        bq_, bk_, bv_ = bq[cols] * scale, bk[cols], bv[cols]
        for h in range(NHL):
            for (pos, mat, bb) in (
                (QPOS[h], wq, bq_),
                (KPOS[h], wk, bk_),
                (VPOS[h], wv, bv_),
            ):
                gi, off = pos
                r0 = gi * 128 + off
                w[:, r0 : r0 + D] = mat[:, h * D : (h + 1) * D]
                bcat[r0 : r0 + D] = bb[h * D : (h + 1) * D]
        bias = np.ascontiguousarray(bcat.reshape(NG, 128).T)
        maskr = np.zeros((128, NT + 1), dtype=np.float32)
        maskr[:, :NT] = attention_mask[b, 0, 0, :].reshape(NT, 128).T
        # partition-major relayout: w_host[p, kc*QKV + n] = w[kc*128+p, n]
        # so the device loads w with 128 contiguous 7.5KB descriptors
        w_host = np.ascontiguousarray(
            w.reshape(KC, 128, QKV).transpose(1, 0, 2).reshape(128, KC * QKV)
        )
        in_maps.append(
            {
                "hsT": np.ascontiguousarray(hidden_states[b].T).astype(np.float16),
                "w": w_host.astype(np.float16),
                "bias": bias,
                "maskr": maskr,
                "ident": np.eye(128, dtype=np.float16),
            }
        )

    nc = _get_nc()
    trace = bool(os.environ.get("BASS_KERNEL_TRACE"))
    res = run_bass_kernel_spmd(nc, in_maps, list(range(8)), trace=trace)
    LAST_EXEC_TIME_NS = res.exec_time_ns

    out = np.empty((B, S, HID), dtype=np.float32)
    for c in range(8):
        b, g = divmod(c, 4)
        ctxa = res.results[c]["ctxa"]  # [3, 65, 2048]
        for hl in range(NHL):
            ctx = ctxa[hl, :D, :] / ctxa[hl, D : D + 1, :]  # [64, 2048]
            out[b, :, g * DG + hl * D : g * DG + (hl + 1) * D] = ctx.T
    return out


# revision 27
# speedup vs baseline: 1.0763x; 1.0180x over previous
"""BERT self-attention Bass kernel for 8 Trainium2 NeuronCores.

Problem: hidden_states [2, 2048, 768], 12 heads x 64 dim, fp32.

Sharding (zero-communication): core c in 0..7 handles batch b = c//4 and
head-group g = c%4 (3 heads). Host pre-lays inputs per core (fp16):
  - hsT   [768, 2048]  hidden[b].T
  - w     [768, 640]   QKV weight columns packed into 5 psum row-groups:
                       g0=[q0|q1] g1=[k0|k1] g2=[q2|v0] g3=[k2|v1] g4=[v2|0]
                       (pairing keeps each head's Q and K partition-aligned;
                       softmax 1/8 folded into Wq)
  - bias  [128, 5]     combined bias per row-group (fp32)
  - maskr [128, 17]    attention_mask[b] column-tiled (col i = keys
                       i*128..i*128+127); col 16 unused
  - ident [128, 128]   identity (PE transposes)

Device schedule (fp16 matmuls, fp32 psum accumulate), fully software-
pipelined so the first attention exp fires ~12us in and the ACT engine
(the 107us exp floor) is never starved afterwards:
  - hsT streams in per (kc-chunk, 512-col block) so the first s-block's
    QKV needs only ~1.7MB of DMA; DMA triggers are spread across the
    sync/gpsimd/scalar HWDGE rings with at most small transfers ahead of
    each ring's first compute (a ring's compute drains its DGE queue).
  - Warm-up matmuls on a memset tile keep the PE busy from engine-init
    so the HAM clock ramp (~2.4GHz after a few us continuous busy) is
    done before attention starts.
  - Prefix: g3-cb0 + g2-cb0 kc-major (k2/q2/v1 for queries+keys 0..511),
    then attention on head 2 begins immediately. All remaining QKV
    units (one unit = one (group, 512-col block), 6 accumulating
    matmuls), the V transposes (PE + DVE exp(mask)-fold into V_aug
    [t, 65] = exp(mask)*[V | 1]) are interleaved into attention-step
    slack with a deadline-derived static placement.
  - per (head, s-block of 512), heads 2,0,1: per t-pair step: 2 scores
    matmuls -> one exp over [128,1024] on ACT (unnormalized, no max-sub:
    scores are O(6) by construction) -> 2 PV matmuls accumulate
    ctxT[65, s] (denominator = ones column). PV emission is pipelined
    one step behind so the PE always has scores work while ACT runs exp.
Host: divide rows 0..63 by row 64, transpose to [s, d], interleave heads.
"""

import os

import numpy as np

import concourse.mybir as mybir
import concourse.tile as tile
from concourse import bacc
from concourse.bass_utils import run_bass_kernel_spmd

F32 = mybir.dt.float32
F16 = mybir.dt.float16

B = 2
S = 2048
HID = 768
NH = 12          # total heads
D = 64           # head dim
NHL = 3          # heads per core
DG = NHL * D     # 192 cols of each W per core
QKV = 640        # packed QKVT row space (5 groups of 128)
KC = HID // 128  # 6 contraction chunks
NG = 5           # psum row-groups of QKVT
GW = [128, 128, 128, 128, 64]     # real rows per group
NT = S // 128    # 16 key tiles
SBW = 512        # s-block width
NSB = S // SBW   # 4 s-blocks
NSTEP = NT // 2  # t-pair steps per s-block
NQ = NT // 4     # vaug quarters (4 t-tiles each)

# (group, offset) per quantity and head
QPOS = [(0, 0), (0, 64), (2, 0)]
KPOS = [(1, 0), (1, 64), (3, 0)]
VPOS = [(2, 64), (3, 64), (4, 0)]
HORDER = [2, 0, 1]  # head 2's tiles are ready first

LAST_EXEC_TIME_NS = None

_CACHED_NC = None


def _build_nc():
    nc = bacc.Bacc("TRN2", target_bir_lowering=False, debug=False, num_devices=8)

    hsT_d = nc.dram_tensor("hsT", [HID, S], F16, kind="ExternalInput")
    # w is host-relaid as [128, KC, 640] (partition-major) so the whole
    # tensor loads with one trigger of 128 contiguous 7.5KB descriptors —
    # DMA throughput is descriptor-bound below ~4KB/descriptor
    w_d = nc.dram_tensor("w", [128, KC * QKV], F16, kind="ExternalInput")
    bias_d = nc.dram_tensor("bias", [128, NG], F32, kind="ExternalInput")
    maskr_d = nc.dram_tensor("maskr", [128, NT + 1], F32, kind="ExternalInput")
    ident_d = nc.dram_tensor("ident", [128, 128], F16, kind="ExternalInput")
    out_d = nc.dram_tensor("ctxa", [NHL, D + 1, S], F32, kind="ExternalOutput")

    with tile.TileContext(nc) as tc:
        with (
            tc.tile_pool(name="const", bufs=1) as cp,
            tc.tile_pool(name="qkvt", bufs=1) as qp,
            tc.tile_pool(name="vaug", bufs=1) as vp,
            tc.tile_pool(name="probs", bufs=3) as pp,
            tc.tile_pool(name="oc", bufs=3) as op,
            tc.tile_pool(name="ps_sc", bufs=2, space="PSUM") as ps_sc,
            tc.tile_pool(name="ps_cx", bufs=2, space="PSUM") as ps_cx,
            tc.tile_pool(name="ps_ac", bufs=2, space="PSUM") as ps_ac,
        ):
            # --- SBUF tiles ---
            w_sb = cp.tile([128, KC, QKV], F16, tag="w")
            w_ap = w_d.ap().rearrange("p (kc n) -> p kc n", kc=KC)
            bias_sb = cp.tile([128, NG], F32, tag="bias")
            maskr_sb = cp.tile([128, NT + 1], F32, tag="maskr")
            ident = cp.tile([128, 128], F16, tag="ident")
            wfsrc = cp.tile([128, 16], F16, tag="wfsrc")
            hs = [
                cp.tile([128, S], F16, tag=f"hsT{kc}", name=f"hsT{kc}")
                for kc in range(KC)
            ]
            qkvt = [
                qp.tile([128, S], F16, tag=f"qkvt{g}", name=f"qkvt{g}")
                for g in range(NG)
            ]
            # K weights per head in [128, S] tiles with the other 64
            # partitions zeroed: scores matmuls then contract over K=128
            # (the zero rows multiply whatever sits in the rhs partitions
            # and contribute nothing)
            ktp = [
                qp.tile([128, S], F16, tag=f"ktp{h}", name=f"ktp{h}")
                for h in range(NHL)
            ]
            vaug = [
                vp.tile([128, NT, D + 1], F16, tag=f"vaug{h}", name=f"vaug{h}")
                for h in range(NHL)
            ]
            em = cp.tile([128, NT], F32, tag="em")

            # --- DMA triggers, spread across all three HWDGE rings ---
            # (each ring's queue sustains only ~110GB/s, so the 4.1MB of
            # input needs all three to land in ~13us). sync: hsT chunks
            # 0,2,4 (no compute on sync). gpsimd: wfsrc memset first (PE
            # warm-up must not wait), then the w thirds and chunk 5.
            # scalar: chunks 1,3 + the small tensors — they all land
            # before ACT's first exp needs its DGE drain.
            nc.gpsimd.memset(wfsrc[:], 1.0)
            for j in range(3):
                nc.gpsimd.dma_start(
                    w_sb[:, 2 * j : 2 * j + 2, :], w_ap[:, 2 * j : 2 * j + 2, :]
                )
            RING = {0: nc.sync, 1: nc.sync, 2: nc.sync, 3: nc.gpsimd,
                    4: nc.sync, 5: nc.gpsimd}
            for kc in range(KC):
                RING[kc].dma_start(
                    hs[kc][:], hsT_d.ap()[kc * 128 : (kc + 1) * 128, :]
                )
            nc.scalar.dma_start(bias_sb[:], bias_d.ap())
            nc.scalar.dma_start(maskr_sb[:], maskr_d.ap())
            nc.scalar.dma_start(ident[:], ident_d.ap())

            # zero fill for ktp[2]: DVE queue head, done long before the
            # h2 scores need it; ktp[0]/ktp[1] zeroes are deferred into
            # interleave closures so they don't delay the prefix copies
            nc.vector.memset(ktp[2][64:128, :], 0.0)

            # --- PE warm-up: keep the PE busy from engine-init onward so
            # the HAM clock ramp completes before real work piles up ---
            def warm(n):
                for _ in range(n):
                    wf = ps_ac.tile([128, SBW], F32, tag="acc", name="wf")
                    nc.tensor.matmul(
                        wf[:16, :16],
                        wfsrc[:],
                        wfsrc[:],
                        start=True,
                        stop=True,
                        skip_group_check=True,
                    )

            # --- QKV units: one unit = (group gi, column block cb), six
            # accumulating matmuls over the kc chunks + psum->sbuf copy
            # (DVE, bias fused) on the last. acc_of overrides the psum
            # accumulator (the prefix borrows the idle sc-pool banks). ---
            KORD = [0, 1, 2, 3, 4, 5]  # expected chunk-arrival order

            def qkv_unit(gi, cb, acc_of=None, act_copy="", defer=None):
                gw = GW[gi]
                st8 = {}

                for i in range(KC):

                    def mm(i=i, kc=KORD[i], gi=gi, cb=cb, gw=gw):
                        if i == 0:
                            st8["acc"] = (
                                acc_of()
                                if acc_of is not None
                                else ps_ac.tile(
                                    [128, SBW], F32, tag="acc", name="acc"
                                )
                            )
                        acc = st8["acc"]
                        nc.tensor.matmul(
                            acc[:gw, :],
                            w_sb[:, kc, gi * 128 : gi * 128 + gw],
                            hs[kc][:, cb * SBW : (cb + 1) * SBW],
                            start=(i == 0),
                            stop=(i == KC - 1),
                        )
                        if i == KC - 1:
                            sl = slice(cb * SBW, (cb + 1) * SBW)

                            def cp_(dst, rows, b0, which=""):
                                if defer is not None:
                                    defer.append(
                                        lambda: cp2_(dst, rows, b0, which)
                                    )
                                    return
                                cp2_(dst, rows, b0, which)

                            def cp2_(dst, rows, b0, which=""):
                                bias_ap = bias_sb[
                                    b0 : b0 + (rows.stop - rows.start),
                                    gi : gi + 1,
                                ]
                                if which in act_copy and which:
                                    # ACT is idle before the first exp —
                                    # run this copy there, in parallel
                                    # with the DVE copies
                                    nc.scalar.activation(
                                        dst[rows, sl],
                                        acc[rows, :],
                                        mybir.ActivationFunctionType.Identity,
                                        bias=bias_ap,
                                    )
                                else:
                                    nc.vector.tensor_scalar_add(
                                        dst[rows, sl],
                                        acc[rows, :],
                                        bias_ap,
                                    )

                            lo, hi = slice(0, 64), slice(64, 128)
                            if gi == 0:  # q0|q1
                                cp_(qkvt[0], slice(0, 128), 0, "q")
                            elif gi == 1:  # k0|k1 -> ktp
                                cp_(ktp[0], lo, 0, "k")
                                cp_(ktp[1], hi, 64, "k")
                            elif gi == 2:  # q2|v0
                                cp_(qkvt[2], slice(0, 128), 0, "q")
                            elif gi == 3:  # k2|v1
                                cp_(ktp[2], lo, 0, "k")
                                cp_(qkvt[3], hi, 64, "v")
                            else:  # v2
                                cp_(qkvt[4], lo, 0, "v")

                    yield mm

            # --- V_aug quarter: 4 PE transposes + DVE exp(mask) fold ---
            def vaug_quarter(h, q):
                def unit(h=h, q=q):
                    ti, off = VPOS[h]
                    vt = qkvt[ti]
                    tp = ps_ac.tile([128, 4 * D], F16, tag="acc", name="tp")
                    for j in range(4):
                        tt = 4 * q + j
                        nc.tensor.transpose(
                            tp[:, j * D : (j + 1) * D],
                            vt[off : off + D, tt * 128 : (tt + 1) * 128],
                            ident[off : off + D, off : off + D],
                        )
                    emq = em[:, 4 * q : 4 * (q + 1)]
                    nc.vector.tensor_tensor(
                        vaug[h][:, 4 * q : 4 * (q + 1), :D],
                        tp[:].rearrange("p (j d) -> p j d", d=D),
                        emq.rearrange("p (j o) -> p j o", o=1)
                        .broadcast_to([128, 4, D]),
                        mybir.AluOpType.mult,
                    )
                    nc.vector.tensor_copy(
                        vaug[h][:, 4 * q : 4 * (q + 1), D : D + 1],
                        emq.rearrange("p (j o) -> p j o", o=1),
                    )

                return unit

            # em[t] = exp(mask_t), folded into V_aug (ACT; before the
            # attention exps in the ACT queue)
            nc.scalar.activation(
                em[:], maskr_sb[:, :NT], mybir.ActivationFunctionType.Exp
            )

            warm(30)

            # --- prefix: ALL FIVE groups of column-block 0 plus three
            # cb1 "floaters", kc-major (the PE chews each chunk faster
            # than the two DMA queues stream them in, so this whole phase
            # is DMA-paced and the HAM clock ramp completes before
            # attention; the floaters soak up the leftover PE idle and
            # empty h2's sb1 production out of the attention steps).
            # Accumulators borrow every idle psum bank: scA = g3|g2,
            # scB = g4|g0, acc-tag = g1 + g2cb1, ctx-tag = g3cb1 + g4cb1.
            scA = ps_sc.tile([128, 2 * SBW], F32, tag="sc", name="scA")
            scB = ps_sc.tile([128, 2 * SBW], F32, tag="sc", name="scB")

            def ctx_acc():
                return ps_cx.tile([128, SBW], F32, tag="ctx", name="ctx")

            g1cp = []  # g1-cb0's copies, deferred past the transition
            pref = [
                list(qkv_unit(3, 0, lambda: scA[:, :SBW], act_copy="v")),
                list(qkv_unit(2, 0, lambda: scA[:, SBW:])),
                list(qkv_unit(4, 0, lambda: scB[:, :SBW])),
                list(qkv_unit(4, 1, lambda: scB[:, SBW:])),
                list(qkv_unit(3, 1, ctx_acc)),
                list(qkv_unit(1, 0, ctx_acc, defer=g1cp)),
            ]
            for i in range(KC):
                for u in pref:
                    u[i]()

            # --- interleave plan: every remaining unit gets an emission
            # window [d0, d1] in global attention steps (96 = 3 heads x 4
            # s-blocks x 8 t-pair steps); closures are spread over the
            # window. Windows front-load just enough to meet each
            # consumer's first-use step (h2 needs all K2/V2 by step 7,
            # q2-sb(k) by step 8k; h0 at steps 32.. needs g0/g1/T0;
            # h1 at 64.. needs only q1 (g0) and T1 which are relaxed). ---
            plan = []

            def add(closures, d0, d1):
                plan.append((list(closures), d0, d1))

            # NOTE: a unit whose output feeds the scores matmul of step s
            # must have d1 <= s-1 (scores of step s are EMITTED before
            # sched[s] runs; a later write would create no dep edge and
            # the scores would read garbage) — and d1 <= s-2 where slack
            # allows, to hide the DVE psum->sbuf copy latency. PV
            # consumers of step st are emitted at step st+1's flush,
            # after sched[st+1].
            def ktp_zero(h, rows, c0):
                def unit(h=h, rows=rows, c0=c0):
                    nc.vector.memset(ktp[h][rows, c0 : c0 + S // 2], 0.0)

                return unit

            add([vaug_quarter(2, 0)], 0, 0)
            add([vaug_quarter(2, 1)], 1, 1)
            add([lambda: [c() for c in g1cp]], 2, 2)
            add(qkv_unit(2, 1), 5, 7)
            add(qkv_unit(3, 2), 0, 2)
            add(qkv_unit(4, 2), 2, 3)
            add([vaug_quarter(2, 2)], 4, 4)
            add(qkv_unit(3, 3), 3, 4)
            add(qkv_unit(4, 3), 4, 5)
            add([vaug_quarter(2, 3)], 6, 6)
            add(qkv_unit(2, 2), 7, 13)
            add(qkv_unit(2, 3), 14, 20)
            add([ktp_zero(0, slice(64, 128), 0),
                 ktp_zero(0, slice(64, 128), S // 2),
                 ktp_zero(1, slice(0, 64), 0),
                 ktp_zero(1, slice(0, 64), S // 2)], 17, 20)
            add(qkv_unit(0, 0), 21, 27)
            add([vaug_quarter(0, 0)], 30, 31)
            add(qkv_unit(1, 1), 31, 32)
            add([vaug_quarter(0, 1)], 33, 33)
            add(qkv_unit(1, 2), 33, 34)
            add([vaug_quarter(0, 2)], 35, 35)
            add(qkv_unit(1, 3), 35, 36)
            add([vaug_quarter(0, 3)], 37, 37)
            add(qkv_unit(0, 1), 37, 38)
            add(qkv_unit(0, 2), 42, 46)
            add(qkv_unit(0, 3), 50, 54)
            add([vaug_quarter(1, 0)], 58, 58)
            add([vaug_quarter(1, 1)], 60, 60)
            add([vaug_quarter(1, 2)], 62, 62)
            add([vaug_quarter(1, 3)], 64, 64)

            sched = [[] for _ in range(96)]
            for closures, d0, d1 in plan:
                span = d1 - d0 + 1
                for i, c in enumerate(closures):
                    g = d0 + (i * span) // len(closures)
                    sched[g].append(c)

            # --- attention ---
            # software-pipelined emission: the PV pair of step k is emitted
            # AFTER exp(k+1), so in the PE's static order the next scores
            # pair runs while ACT is busy and ACT never waits on the PE.
            pending = [None]  # (h, ctx, st, pr, oc_args) awaiting PV

            def flush_pending():
                if pending[0] is None:
                    return
                (ph, pctx, pst, ppr, poc) = pending[0]
                for half in range(2):
                    tt = pst * 2 + half
                    nc.tensor.matmul(
                        pctx[: D + 1, :],
                        vaug[ph][:, tt, :],
                        ppr[:, half * SBW : (half + 1) * SBW],
                        start=(tt == 0),
                        stop=(tt == NT - 1),
                    )
                if poc is not None:
                    h_, s0_ = poc
                    oc = op.tile([128, SBW], F32, tag="oc", name="oc")
                    if h_ == HORDER[-1] and s0_ == S - SBW:
                        # final s-block: copy + DMA in halves so the DMA
                        # descriptor-gen overlaps the second copy half
                        for c0 in (0, SBW // 2):
                            hsl = slice(c0, c0 + SBW // 2)
                            nc.vector.tensor_copy(
                                oc[: D + 1, hsl], pctx[: D + 1, hsl]
                            )
                            nc.sync.dma_start(
                                out_d.ap()[h_, :, s0_ + c0 : s0_ + c0 + SBW // 2],
                                oc[: D + 1, hsl],
                            )
                    else:
                        nc.vector.tensor_copy(oc[: D + 1, :], pctx[: D + 1, :])
                        nc.sync.dma_start(
                            out_d.ap()[h_, :, s0_ : s0_ + SBW],
                            oc[: D + 1, :],
                        )

            QTILE = [0, 0, 2]  # rhs tile per head (full 128 partitions)
            gstep = 0
            for h in HORDER:
                qt, kt = qkvt[QTILE[h]], ktp[h]
                for sbk in range(NSB):
                    s0 = sbk * SBW
                    ctx = ps_cx.tile([128, SBW], F32, tag="ctx", name="ctx")
                    for st in range(NSTEP):  # t-pair steps
                        sc = ps_sc.tile(
                            [128, 2 * SBW], F32, tag="sc", name="sc"
                        )
                        for half in range(2):
                            tt = st * 2 + half
                            nc.tensor.matmul(
                                sc[:, half * SBW : (half + 1) * SBW],
                                kt[:, tt * 128 : (tt + 1) * 128],
                                qt[:, s0 : s0 + SBW],
                                start=True,
                                stop=True,
                            )
                        for c in sched[gstep]:
                            c()
                        pr = pp.tile([128, 2 * SBW], F16, tag="pr", name="pr")
                        nc.scalar.activation(
                            pr[:], sc[:], mybir.ActivationFunctionType.Exp
                        )
                        flush_pending()
                        pending[0] = (
                            h,
                            ctx,
                            st,
                            pr,
                            (h, s0) if st == NSTEP - 1 else None,
                        )
                        gstep += 1
            flush_pending()

    nc.compile()
    return nc


def _get_nc():
    global _CACHED_NC
    if _CACHED_NC is None:
        _CACHED_NC = _build_nc()
    return _CACHED_NC


def kernel(
    hidden_states, attention_mask, Wq, bq, Wk, bk, Wv, bv
) -> np.ndarray:
    global LAST_EXEC_TIME_NS
    hidden_states = np.asarray(hidden_states, dtype=np.float32)
    attention_mask = np.asarray(attention_mask, dtype=np.float32)
    Wq = np.asarray(Wq, dtype=np.float32)
    Wk = np.asarray(Wk, dtype=np.float32)
    Wv = np.asarray(Wv, dtype=np.float32)
    bq = np.asarray(bq, dtype=np.float32)
    bk = np.asarray(bk, dtype=np.float32)
    bv = np.asarray(bv, dtype=np.float32)

    scale = 1.0 / np.sqrt(np.float32(D))

    in_maps = []
    for c in range(8):
        b, g = divmod(c, 4)
        cols = slice(g * DG, (g + 1) * DG)
        wq = Wq[:, cols] * scale
        wk = Wk[:, cols]
        wv = Wv[:, cols]
        w = np.zeros((HID, QKV), dtype=np.float32)
        bcat = np.zeros(QKV, dtype=np.float32)
        bq_, bk_, bv_ = bq[cols] * scale, bk[cols], bv[cols]
        for h in range(NHL):
            for (pos, mat, bb) in (
                (QPOS[h], wq, bq_),
                (KPOS[h], wk, bk_),
                (VPOS[h], wv, bv_),
            ):
                gi, off = pos
                r0 = gi * 128 + off
                w[:, r0 : r0 + D] = mat[:, h * D : (h + 1) * D]
                bcat[r0 : r0 + D] = bb[h * D : (h + 1) * D]
        bias = np.ascontiguousarray(bcat.reshape(NG, 128).T)
        maskr = np.zeros((128, NT + 1), dtype=np.float32)
        maskr[:, :NT] = attention_mask[b, 0, 0, :].reshape(NT, 128).T
        # partition-major relayout: w_host[p, kc*QKV + n] = w[kc*128+p, n]
        # so the device loads w with 128 contiguous 7.5KB descriptors
        w_host = np.ascontiguousarray(
            w.reshape(KC, 128, QKV).transpose(1, 0, 2).reshape(128, KC * QKV)
        )
        in_maps.append(
            {
                "hsT": np.ascontiguousarray(hidden_states[b].T).astype(np.float16),
                "w": w_host.astype(np.float16),
                "bias": bias,
                "maskr": maskr,
                "ident": np.eye(128, dtype=np.float16),
            }
        )

    nc = _get_nc()
    trace = bool(os.environ.get("BASS_KERNEL_TRACE"))
    res = run_bass_kernel_spmd(nc, in_maps, list(range(8)), trace=trace)
    LAST_EXEC_TIME_NS = res.exec_time_ns

    out = np.empty((B, S, HID), dtype=np.float32)
    for c in range(8):
        b, g = divmod(c, 4)
        ctxa = res.results[c]["ctxa"]  # [3, 65, 2048]
        for hl in range(NHL):
            ctx = ctxa[hl, :D, :] / ctxa[hl, D : D + 1, :]  # [64, 2048]
            out[b, :, g * DG + hl * D : g * DG + (hl + 1) * D] = ctx.T
    return out
